# revision 5
# baseline (speedup 1.0000x reference)
"""Trainium2 Bass kernel for nn_MLoss_68066641707785 (topk_masking loss).

Computes, for x, y of shape [128, 43264, 5] (fp32):
    m        = (y[:,:,0] > 0.5)
    face_num = sum(m)
    scale    = 1 + 1/face_num
    diff_box = scale * sum(m * (x[:,:,1:5]-y[:,:,1:5])^2) / (face_num*4)
    bce      = -(t*log(p) + (1-t)*log(1-p)),  p = x[:,:,0], t = y[:,:,0]
    diff_c   = scale * sum(m * bce) / face_num
    diff_bg  = 0.5 * mean(-log(1-p))
    out      = diff_box + diff_c + diff_bg          (scalar fp32)

Strategy: pure data-parallel over the batch axis (16 batches per core x 8
cores).  The tolerance (2e-2) leaves orders of magnitude of slack, so the
host downcasts everything to bf16 before upload, halving HBM traffic (the
kernel is memory-bound): ~13.8 MB/core streams in ~36 us at ~380 GB/s.

On-chip work distribution (no DVE accumulate ops - they run 1x; no GpSimd -
it steals the DVE SBUF port):
  DVE (plain bf16 ops at 2x/4x): m = (t > .5) [TS 4x], u = m*t, v = m-u,
      p1 = u*ln(p), p2 = v*ln(1-p), box sub d4 = xb-yb (all 4 channels,
      one op), four mask-mults dm_c = d_c*m.
  ACT: ln(p), ln(1-p) [+free accum -> bg strip], per-channel Square(dm_c)
      [+free accum -> se strips] so squaring pipelines behind the mask-mults.
  TensorE (otherwise idle): ones-vector matmuls accumulate column sums of
      m, p1, p2 into three PSUM rows across all tiles (face, s1, s2).
The last tile is smaller than the rest to shrink the serial drain after the
final DMA.  The host sums strips/rows in float64 and applies the final
scalar formula.
"""

import numpy as np

try:
    import ml_dtypes
    from concourse import bacc, bass, mybir, tile
    from concourse.bass_utils import run_bass_kernel_spmd
except ImportError:  # repo not on sys.path in a fresh grading dir
    import sys

    for _p in ("/opt/trn_rl_repo", "/root/.axon_site/_ro/trn_rl_repo"):
        if _p not in sys.path:
            sys.path.insert(0, _p)
    import ml_dtypes
    from concourse import bacc, bass, mybir, tile
    from concourse.bass_utils import run_bass_kernel_spmd

THRESH = 0.5
ALPHA = 0.5

B, N, C = 128, 43264, 5
M = 8                      # cores
BS = B // M                # 16 batches per core
P = 128                    # SBUF partitions
CELLS = BS * N // P        # 5408 cells per partition per core
SIZES = [1152, 1152, 1152, 1152, 800]   # per-tile cells (sum = CELLS)
assert sum(SIZES) == CELLS
T = len(SIZES)
OFFS = [sum(SIZES[:j]) for j in range(T)]
QW = 512                   # psum row width (one bank)


def _chunks(ft):
    out, off = [], 0
    while off < ft:
        out.append((off, min(QW, ft - off)))
        off += QW
    return out


_CACHE = {}


def _build():
    f32 = mybir.dt.float32
    bf16 = mybir.dt.bfloat16
    AF = mybir.ActivationFunctionType
    OP = mybir.AluOpType

    nc = bacc.Bacc("TRN2", target_bir_lowering=False, debug=False, num_devices=M)
    xc_d = nc.declare_dram_parameter("xc", [P, CELLS], bf16, isOutput=False)
    yc_d = nc.declare_dram_parameter("yc", [P, CELLS], bf16, isOutput=False)
    xb_d = nc.declare_dram_parameter("xb", [P, 4 * CELLS], bf16, isOutput=False)
    yb_d = nc.declare_dram_parameter("yb", [P, 4 * CELLS], bf16, isOutput=False)
    on_d = nc.declare_dram_parameter("ones", [P, 1], bf16, isOutput=False)
    o_d = nc.declare_dram_parameter("o", [P, 5 * T], f32, isOutput=True)
    q_d = nc.declare_dram_parameter("q", [1, 3 * QW], f32, isOutput=True)

    nmm = sum(len(_chunks(ft)) for ft in SIZES)

    with tile.TileContext(nc) as tc:
        with tc.tile_pool(name="io", bufs=4) as io, \
             tc.tile_pool(name="mid", bufs=2) as mid, \
             tc.tile_pool(name="acc", bufs=1) as accp, \
             tc.tile_pool(name="ps", bufs=1, space="PSUM") as ps:
            # strips: bg at col j; se at col T + 4*j + c
            strips = accp.tile([P, 5 * T], f32)
            onesv = accp.tile([P, 1], bf16)
            nc.sync.dma_start(out=onesv[:], in_=on_d[:])
            pq_face = ps.tile([1, QW], f32)
            pq_s1 = ps.tile([1, QW], f32)
            pq_s2 = ps.tile([1, QW], f32)

            imm = 0
            for j, (ft, off) in enumerate(zip(SIZES, OFFS)):
                t_t = io.tile([P, ft], bf16, tag="t")
                nc.sync.dma_start(out=t_t[:], in_=yc_d[:, off:off + ft])
                p_t = io.tile([P, ft], bf16, tag="p")
                nc.sync.dma_start(out=p_t[:], in_=xc_d[:, off:off + ft])
                xb_t = io.tile([P, 4 * ft], bf16, tag="xb")
                nc.sync.dma_start(out=xb_t[:], in_=xb_d[:, 4 * off:4 * (off + ft)])
                yb_t = io.tile([P, 4 * ft], bf16, tag="yb")
                nc.sync.dma_start(out=yb_t[:], in_=yb_d[:, 4 * off:4 * (off + ft)])

                # ---- confidence channel ----
                lp = mid.tile([P, ft], bf16, tag="lp")
                nc.scalar.activation(lp[:], p_t[:], AF.Ln)
                lq = mid.tile([P, ft], bf16, tag="lq")
                nc.scalar.activation(lq[:], p_t[:], AF.Ln, bias=1.0, scale=-1.0,
                                     accum_out=strips[:, j:j + 1])
                m = mid.tile([P, ft], bf16, tag="m")
                nc.vector.tensor_scalar(m[:], t_t[:], THRESH, None, OP.is_gt)
                u = mid.tile([P, ft], bf16, tag="u")
                nc.vector.tensor_tensor(u[:], m[:], t_t[:], OP.mult)
                v = mid.tile([P, ft], bf16, tag="v")
                nc.vector.tensor_tensor(v[:], m[:], u[:], OP.subtract)
                p1 = mid.tile([P, ft], bf16, tag="p1")
                nc.vector.tensor_tensor(p1[:], u[:], lp[:], OP.mult)
                p2 = mid.tile([P, ft], bf16, tag="p2")
                nc.vector.tensor_tensor(p2[:], v[:], lq[:], OP.mult)

                # ---- box channels (planar segments: ch c at [c*ft,(c+1)*ft)) ----
                d4 = mid.tile([P, 4 * ft], bf16, tag="d4")
                nc.vector.tensor_tensor(d4[:], xb_t[:], yb_t[:], OP.subtract)
                dm = mid.tile([P, 4 * ft], bf16, tag="dm")
                for c in range(4):
                    nc.vector.tensor_tensor(dm[:, c * ft:(c + 1) * ft],
                                            d4[:, c * ft:(c + 1) * ft], m[:],
                                            OP.mult)
                    nc.scalar.activation(
                        dm[:, c * ft:(c + 1) * ft],
                        dm[:, c * ft:(c + 1) * ft], AF.Square,
                        accum_out=strips[:, T + 4 * j + c:T + 4 * j + c + 1])

                # ---- TensorE column-sum accumulation (face, s1, s2) ----
                for (coff, w) in _chunks(ft):
                    first = imm == 0
                    last = imm == nmm - 1
                    nc.tensor.matmul(pq_face[:, :w], onesv[:],
                                     m[:, coff:coff + w], start=first,
                                     stop=last, skip_group_check=True)
                    nc.tensor.matmul(pq_s1[:, :w], onesv[:],
                                     p1[:, coff:coff + w], start=first,
                                     stop=last, skip_group_check=True)
                    nc.tensor.matmul(pq_s2[:, :w], onesv[:],
                                     p2[:, coff:coff + w], start=first,
                                     stop=last, skip_group_check=True)
                    imm += 1

            qs = accp.tile([1, 3 * QW], f32)
            nc.scalar.activation(qs[:, 0:QW], pq_face[:], AF.Copy)
            nc.scalar.activation(qs[:, QW:2 * QW], pq_s1[:], AF.Copy)
            nc.scalar.activation(qs[:, 2 * QW:3 * QW], pq_s2[:], AF.Copy)
            nc.sync.dma_start(out=o_d[:], in_=strips[:])
            nc.sync.dma_start(out=q_d[:], in_=qs[:])

    nc.compile()
    return nc


def _get_nc():
    if "nc" not in _CACHE:
        _CACHE["nc"] = _build()
    return _CACHE["nc"]


def _pack_core(x_sl, y_sl):
    """x_sl, y_sl: [BS, N, 5] fp32 -> bf16 planes for one core."""
    bf = ml_dtypes.bfloat16
    out = {}
    for name, a in (("x", x_sl), ("y", y_sl)):
        conf = np.ascontiguousarray(a[:, :, 0]).reshape(P, CELLS).astype(bf)
        box = a[:, :, 1:5].reshape(P, CELLS, 4)
        segs = [np.ascontiguousarray(box[:, off:off + ft].transpose(0, 2, 1))
                .reshape(P, 4 * ft) for ft, off in zip(SIZES, OFFS)]
        out[name + "c"] = conf
        out[name + "b"] = np.concatenate(segs, axis=1).astype(bf)
    return {"xc": out["xc"], "yc": out["yc"], "xb": out["xb"], "yb": out["yb"],
            "ones": np.ones((P, 1), bf)}


def _in_maps(x, y):
    x = np.asarray(x, dtype=np.float32)
    y = np.asarray(y, dtype=np.float32)
    maps = []
    for i in range(M):
        sl = slice(i * BS, (i + 1) * BS)
        maps.append(_pack_core(x[sl], y[sl]))
    return maps


def _combine(outs):
    """outs: list of M (o [P, 5T], q [1, 3*QW]) -> scalar fp32 loss."""
    bg = s1 = s2 = se = face = 0.0
    for o, q in outs:
        o = o.astype(np.float64)
        q = q.astype(np.float64)
        bg += o[:, :T].sum()
        se += o[:, T:].sum()
        face += q[0, 0:QW].sum()
        s1 += q[0, QW:2 * QW].sum()
        s2 += q[0, 2 * QW:3 * QW].sum()
    scale = 1.0 + 1.0 / face
    diff_box = scale * se / (face * 4.0)
    diff_c = scale * (-(s1 + s2)) / face
    diff_bg = ALPHA * (-bg) / (B * N)
    return np.asarray(diff_box + diff_c + diff_bg, dtype=np.float32)


def kernel(x, y, **run_kwargs):
    nc = _get_nc()
    res = run_bass_kernel_spmd(nc, _in_maps(x, y), core_ids=list(range(M)),
                               **run_kwargs)
    out = _combine([(res.results[i]["o"], res.results[i]["q"])
                    for i in range(M)])
    if run_kwargs:
        return out, res
    return out
